# revision 37
# baseline (speedup 1.0000x reference)
"""Multi-head self-attention (B=2, N=2048, C=1024, H=16) on 8 trn2 NeuronCores.

Sharding: core i computes heads {2i, 2i+1} for both batches (head-parallel
attention); a head-split pair of 8-way AllToAlls redistributes attention
outputs so core i holds the full channel dim for output rows
(b = i//4, seq chunk 512*(i%4)); each core then computes its own 512-row
slice of the output projection.

All matmuls present a full 128x128 stationary tile to the PE array so the
HAM clock gate holds the 2.4 GHz state (half-array matmuls run at 1.2 GHz):
 - scores: k is stored zero-spread per head (kTz{h} has head h's 64 rows,
   zeros elsewhere) so K=128 against the packed q tile adds 0*q_other.
 - att@v: stationary is [v_h | 64 ones columns]; psum rows 64:128 come out
   as the softmax denominator replicated across 64 partitions; one copy
   releases the accumulator and the reciprocal runs on a DMA-packed
   [64,32] repack (a flat [64,2048] DVE reciprocal costs 13us).
The exp(scores) stream on the Scalar engine is the critical path; the nk
loop emits next-iteration score matmuls between att@v matmuls so ACT never
waits on PE.
"""

import numpy as np
import ml_dtypes
import bass_rust

import concourse.bass as bass
import concourse.mybir as mybir
import concourse.tile as tile
from concourse.bass_utils import run_bass_kernel_spmd

B, N, C = 2, 2048, 1024
H = 16
D = C // H           # 64
W = 8                # cores
HL = 2               # heads per core
P = 128
KT = C // P          # 8 k-tiles over channels
NQC = N // 512       # 4 query chunks of 512 per batch
NK = N // P          # 16 key tiles per batch
SCALE = float(D) ** -0.5

F32 = mybir.dt.float32
BF16 = mybir.dt.bfloat16
BF = ml_dtypes.bfloat16


_RING_INSTS = (
    mybir.InstDMACopy, mybir.InstDMA, mybir.InstTensorLoad, mybir.InstTensorSave,
    mybir.InstDmaTransposeAnt, mybir.InstDMAGatherAnt, mybir.InstDMAScatterAddAnt,
    mybir.InstCollectiveCompute,
)


def _split_multiwait(nc: bass.Bass, gate_sems: dict) -> None:
    """This toolchain's walrus codegen accepts at most ONE sync wait per
    instruction, but the Tile scheduler attaches several.

    Compute/CTRL instructions: move all but the last wait onto EventSemaphore
    instructions inserted just before them on the same engine stream (engine
    sequencers execute in order, so the stall transfers).

    DMA / collective instructions are processed by the DGE ring / TOPSP, which
    a preceding stream stall does not reliably gate. For those, the inserted
    EventSemaphores absorb ALL original waits and the last one increments a
    per-engine gate semaphore; the ring instruction then carries the single
    gate wait."""
    ctr = 0
    counts: dict[int, int] = {}
    for fn in nc.m.functions:
        for bb in fn.blocks:
            out = []
            changed = False
            for ins in bb.instructions:
                si = ins.sync_info
                if si is None or len(si.on_wait) <= 1:
                    out.append(ins)
                    continue
                changed = True
                waits = list(si.on_wait)
                eng = ins.engine
                if isinstance(ins, _RING_INSTS):
                    h = gate_sems[eng]
                    cnt = counts.get(h.num, 0) + 1
                    counts[h.num] = cnt
                    for j, w in enumerate(waits):
                        ctr += 1
                        ev = mybir.InstEventSemaphore(
                            name=f"gate-ev-{ctr}", engine=eng)
                        upd = []
                        if j == len(waits) - 1:
                            upd = [bass_rust.SyncUpdate(
                                sync_type="semaphore", id=h.num, ant_name=h.name,
                                update_mode="sem-inc", update_value=1,
                                update_reg=None)]
                        ev.sync_info = bass_rust.SyncInfo(on_wait=[w], on_update=upd)
                        out.append(ev)
                    ins.sync_info = bass_rust.SyncInfo(
                        on_wait=[bass_rust.SyncWait(
                            sync_type="semaphore", id=h.num, ant_name=h.name,
                            wait_mode="sem-ge-imm", wait_value=cnt,
                            wait_reg=None)],
                        on_update=list(si.on_update),
                    )
                else:
                    for w in waits[:-1]:
                        ctr += 1
                        ev = mybir.InstEventSemaphore(
                            name=f"gate-ev-{ctr}", engine=eng)
                        ev.sync_info = bass_rust.SyncInfo(on_wait=[w], on_update=[])
                        out.append(ev)
                    ins.sync_info = bass_rust.SyncInfo(
                        on_wait=[waits[-1]], on_update=list(si.on_update)
                    )
                out.append(ins)
            if changed:
                bb.instructions = out


def _build_nc() -> bass.Bass:
    nc = bass.Bass()
    gate_sems = {
        e: nc.alloc_semaphore(f"mw_gate_{i}")
        for i, e in enumerate([
            mybir.EngineType.SP, mybir.EngineType.Pool,
            mybir.EngineType.Activation, mybir.EngineType.PE,
            mybir.EngineType.DVE,
        ])
    }

    # DRAM parameters (bf16 compute inputs prepared host-side)
    xT = nc.declare_dram_parameter("xT", [B * NQC, P, KT, 512], BF16, isOutput=False)
    # qkv/proj weights pre-transposed host-side to the SBUF layout
    wq = nc.declare_dram_parameter("wq", [P, KT, P], BF16, isOutput=False)
    wk = nc.declare_dram_parameter("wk", [P, KT, P], BF16, isOutput=False)
    wv = nc.declare_dram_parameter("wv", [P, KT, P], BF16, isOutput=False)
    bq = nc.declare_dram_parameter("bq", [P, 1], F32, isOutput=False)   # pre-scaled
    bk = nc.declare_dram_parameter("bk", [P, 1], F32, isOutput=False)
    bvr = nc.declare_dram_parameter("bvr", [P, P], F32, isOutput=False)  # replicated
    wp = nc.declare_dram_parameter("wp", [P, KT, C], BF16, isOutput=False)
    bp = nc.declare_dram_parameter("bp", [P, KT], F32, isOutput=False)  # [p, mtile]
    out = nc.declare_dram_parameter("out", [C, 512], BF16, isOutput=True)

    from contextlib import ExitStack
    with tile.TileContext(nc) as tc:
        with (
            tc.tile_pool(name="persist", bufs=1) as pp,
            tc.tile_pool(name="work", bufs=3) as wk_pool,
            tc.tile_pool(name="norm", bufs=2) as norm_pool,
            tc.tile_pool(name="dram", bufs=1, space="DRAM") as dram,
            ExitStack() as phase_ctx,
        ):
            psp = phase_ctx.enter_context(
                tc.tile_pool(name="psum", bufs=2, space="PSUM"))
            import contextlib
            psq_ctx = contextlib.ExitStack()
            psq = psq_ctx.enter_context(
                tc.tile_pool(name="psumq", bufs=1, space="PSUM"))
            # ---- persistent SBUF loads ----
            xT_t = [pp.tile([P, KT, 512], BF16, tag=f"xT{j}", name=f"xT_{j}")
                    for j in range(B * NQC)]

            def load_x_chunk(j, splits=1):
                # quarter-split early chunks so their descriptors spread
                # over several DMA queues and land sooner
                step = KT // splits
                for s in range(splits):
                    nc.sync.dma_start(
                        xT_t[j][:, s * step: (s + 1) * step, :],
                        xT[j][:, s * step: (s + 1) * step, :])

            # qkv weights arrive host-pre-transposed as [p, kt*m] so each
            # DMA is 128 descriptors of one contiguous 2KB partition row
            # (the on-the-fly rearrange costs 1024 x 256B descriptors).
            # xT[0] is issued first: it is 4x larger than wq and equally
            # critical for the first matmul.
            load_x_chunk(0, splits=4)
            wq_sb = pp.tile([P, KT, P], BF16, tag="wq")
            nc.sync.dma_start(wq_sb[:], wq[:])
            bq_sb = pp.tile([P, 1], F32, tag="bq")
            nc.sync.dma_start(bq_sb[:], bq[:])
            load_x_chunk(1, splits=4)
            wk_sb = pp.tile([P, KT, P], BF16, tag="wk")
            nc.sync.dma_start(wk_sb[:], wk[:])
            bk_sb = pp.tile([P, 1], F32, tag="bk")
            nc.sync.dma_start(bk_sb[:], bk[:])
            wv_sb = pp.tile([P, KT, P], BF16, tag="wv")
            nc.sync.dma_start(wv_sb[:], wv[:])
            bvr_sb = pp.tile([P, P], F32, tag="bvr")
            nc.sync.dma_start(bvr_sb[:], bvr[:])
            # preload the exp activation table during the DMA phase: the lazy
            # ACT_TABLE_LOAD (~2.7us) otherwise lands inside the attention
            # pipeline fill
            warm_exp = pp.tile([P, 1], F32, tag="warm_exp")
            nc.scalar.activation(
                warm_exp[:], bq_sb[:], mybir.ActivationFunctionType.Exp)

            # persistent activations, one tile per (batch, 512-chunk) so
            # consumers wait only on the chunk they read.  q is packed
            # (head 2i rows 0:64, head 2i+1 rows 64:128); k is stored
            # zero-spread per head so score matmuls can contract K=128.
            qT_t = [pp.tile([P, 512], BF16, tag=f"qT{j}", name=f"qT_{j}")
                    for j in range(B * NQC)]
            kTz_t = [
                [pp.tile([P, 512], BF16, tag=f"kTz{h}_{j}", name=f"kTz{h}_{j}")
                 for j in range(B * NQC)]
                for h in range(HL)
            ]
            # v_ext: [seq128, b, seqtile, head, 128]; cols 0:D hold v, cols
            # D:128 are ones so the av matmul is a full 128-wide stationary
            # operand and psum rows D:128 accumulate the softmax denominator
            # (replicated across 64 partitions).
            v_sb = pp.tile([P, B, NK, HL, P], BF16, tag="v")
            # proj inputs, one per a2a phase; unfilled head-half stays zero
            rx_sb = [pp.tile([P, KT, 512], BF16, tag=f"rx{h}", name=f"rx_{h}")
                     for h in range(HL)]

            a2a_in = [dram.tile([W * D, 512], BF16, tag=f"a2a_in_{h}",
                                name=f"a2a_in_{h}") for h in range(HL)]
            a2a_out = [dram.tile([W * D, 512], BF16, tag=f"a2a_out_{h}",
                                 name=f"a2a_out_{h}") for h in range(HL)]

            # ---- phase 1: qkv ----
            load_x_chunk(2, splits=2)
            load_x_chunk(3, splits=2)
            for j in range(4, B * NQC):
                load_x_chunk(j)
            # proj weights are only needed at the end — load them after the
            # critical-path xT chunks (same FIFO DMA ring)
            wp_sb = pp.tile([P, KT, C], BF16, tag="wp")
            nc.sync.dma_start(wp_sb[:], wp[:])
            bp_sb = pp.tile([P, KT], F32, tag="bp")
            nc.sync.dma_start(bp_sb[:], bp[:])
            # j-outer accumulation: each (b, chunk) finishes its 8-kt psum
            # accumulation then drains while the next chunk's matmuls run.
            # psum tags are laid out so no chunk ever waits a drain: q on
            # tags 0-7, k/v rotate through 4-7.
            def emit_q(j):
                ps = psq.tile([P, 512], F32, tag=f"qk{'AB'[j % 2]}", name=f"ps_q_{j}")
                for kt in range(KT):
                    nc.tensor.matmul(
                        ps[:], wq_sb[:, kt], xT_t[j][:, kt, :],
                        start=(kt == 0), stop=(kt == KT - 1),
                    )
                nc.vector.tensor_scalar(
                    qT_t[j][:], ps[:],
                    SCALE, bq_sb[:], mybir.AluOpType.mult, mybir.AluOpType.add,
                )

            def emit_k(j, tag):
                ps = psq.tile([P, 512], F32, tag=tag, name=f"ps_k_{j}")
                for kt in range(KT):
                    nc.tensor.matmul(
                        ps[:], wk_sb[:, kt], xT_t[j][:, kt, :],
                        start=(kt == 0), stop=(kt == KT - 1),
                    )
                nc.vector.tensor_scalar_add(
                    kTz_t[0][j][0:D, :], ps[0:D, :], bk_sb[0:D],
                )
                nc.vector.tensor_scalar_add(
                    kTz_t[1][j][D:P, :], ps[D:P, :], bk_sb[D:P],
                )
                nc.vector.memset(kTz_t[0][j][D:P, :], 0.0)
                nc.vector.memset(kTz_t[1][j][0:D, :], 0.0)

            def emit_v(b, st):
                tag = f"qk{'AB'[st % 2]}"
                ps = psq.tile([P, P], F32, tag=tag, name=f"ps_v_{b}_{st}")
                xt = xT_t[4 * b + st // 4]
                so = P * (st % 4)
                for kt in range(KT):
                    nc.tensor.matmul(
                        ps[:], xt[:, kt, so: so + P], wv_sb[:, kt],
                        start=(kt == 0), stop=(kt == KT - 1),
                    )
                nc.vector.tensor_tensor(
                    v_sb[:, b, st, :, 0:D],
                    ps.rearrange("p (h d) -> p h d", h=HL),
                    bvr_sb.rearrange("p (h d) -> p h d", h=HL),
                    mybir.AluOpType.add,
                )

            # ---- attention emission helpers (psp is already open, below
            # the qkv rings, so the first head-phase can pre-run scores+exp
            # on the idle ACT engine while late qkv still computes) ----
            def emit_scores_half(b, h, nk, t):
                """Scores for key tile nk, query chunks 2t..2t+1 -> one
                [128, 1024] psum tile.  K=128 against packed q: the zero
                rows of kTz contribute 0 * q_other."""
                ps = psp.tile([P, 1024], F32, tag="s", name=f"ps_s_{b}_{h}_{nk}_{t}")
                kz = kTz_t[h][4 * b + nk // 4]
                ko = P * (nk % 4)
                for i in range(2):
                    c = 2 * t + i
                    nc.tensor.matmul(
                        ps[:, 512 * i: 512 * (i + 1)],
                        kz[:, ko: ko + P],
                        qT_t[4 * b + c][:],
                        start=True, stop=True,
                    )
                return ps

            EXP_BUFS = 13
            PD = 7  # (h0,b0) score/exp pre-run depth in key tiles

            def emit_exp(b, h, nk, t):
                ps = emit_scores_half(b, h, nk, t)
                e = wk_pool.tile([P, 1024], BF16, tag="exp", bufs=EXP_BUFS,
                                 name=f"exp_{b}_{h}_{nk}_{t}")
                nc.scalar.activation(
                    e[:], ps[:], mybir.ActivationFunctionType.Exp)
                return e

            pre_exps = {}

            def pre_nk(nk):
                for t in range(2):
                    pre_exps[(nk, t)] = emit_exp(0, 0, nk, t)

            for j in range(8):
                emit_q(j)
            for j in range(4):
                emit_k(j, f"qk{'CD'[j % 2]}")
            nc.vector.memset(v_sb[:, :, :, :, D:P], 1.0)
            for st in range(NK):
                emit_v(0, st)
            for j in range(4, 8):
                emit_k(j, f"qk{'CD'[j % 2]}")
                pre_nk(j - 4)
            for st in range(NK):
                emit_v(1, st)
                if st < PD - 4:
                    pre_nk(4 + st)
            # qkv psum rings retire; ps_o takes their banks
            psq_ctx.close()
            psp2 = phase_ctx.enter_context(
                tc.tile_pool(name="psum2", bufs=1, space="PSUM"))

            nc.vector.memset(rx_sb[0][D:P, :, :], 0.0)
            nc.vector.memset(rx_sb[1][0:D, :, :], 0.0)

            # ---- phase 2: attention, h-outer so each head-phase feeds an
            # AllToAll that overlaps the next phase's compute ----
            for h in range(HL):
                for b in range(B):
                    ps_o = psp2.tile([P, N], F32, tag="o", name=f"ps_o_{b}_{h}")
                    first = (h == 0 and b == 0)
                    pipe = pre_exps if first else {}
                    pd = PD if first else 1
                    if not first:
                        for t in range(2):
                            pipe[(0, t)] = emit_exp(b, h, 0, t)
                    for nk in range(NK):
                        for t in range(2):
                            if nk + pd < NK:
                                pipe[(nk + pd, t)] = emit_exp(b, h, nk + pd, t)
                            e = pipe.pop((nk, t))
                            for i in range(2):
                                c = 2 * t + i
                                nc.tensor.matmul(
                                    ps_o[:, 512 * c: 512 * (c + 1)],
                                    v_sb[:, b, nk, h],
                                    e[:, 512 * i: 512 * (i + 1)],
                                    start=(nk == 0), stop=(nk == NK - 1),
                                )
                    # normalize rows 0:D by the replicated denominator in
                    # rows D:P.  One full-tile copy releases ps_o; the
                    # reciprocal is a Newton-Raphson pass in 3 plain DVE ops
                    # (seed y0_bits = K - d_bits via xor/add; the NR step
                    # emits -1/den and the sign is folded into host-negated
                    # proj weights).  Outputs land at base partition 0 so the
                    # final multiply is all-SBUF base-aligned.
                    o_full = norm_pool.tile([P, N], F32, tag="ofull", bufs=1,
                                            name=f"ofull_{b}_{h}")
                    nc.vector.tensor_copy(o_full[:], ps_o[:])
                    y0 = norm_pool.tile([D, N], F32, tag="y0", bufs=1,
                                        name=f"y0_{b}_{h}")
                    e_t = norm_pool.tile([D, N], F32, tag="e_t", bufs=1,
                                         name=f"e_t_{b}_{h}")
                    nc.vector.tensor_scalar(
                        e_t[:].bitcast(mybir.dt.int32),
                        ps_o[D:P, :].bitcast(mybir.dt.int32),
                        -1, None, mybir.AluOpType.bitwise_xor,
                    )
                    nc.vector.tensor_scalar(
                        y0[:].bitcast(mybir.dt.int32),
                        e_t[:].bitcast(mybir.dt.int32),
                        0x7EF311C4, None, mybir.AluOpType.add,
                    )
                    nc.vector.tensor_tensor(
                        e_t[:], ps_o[D:P, :], y0[:], mybir.AluOpType.mult,
                    )
                    ny1 = norm_pool.tile([D, N], F32, tag="ny1", bufs=1,
                                         name=f"ny1_{b}_{h}")
                    nc.vector.scalar_tensor_tensor(
                        ny1[:], e_t[:], 2.0, y0[:],
                        mybir.AluOpType.subtract, mybir.AluOpType.mult,
                    )
                    o_sb = norm_pool.tile([D, N], BF16, tag="osb", bufs=1,
                                          name=f"osb_{b}_{h}")
                    nc.vector.tensor_tensor(
                        o_sb[:], o_full[0:D, :], ny1[:], mybir.AluOpType.mult,
                    )
                    nc.sync.dma_start(
                        a2a_in[h].rearrange("(j r) n -> r j n", r=D)[
                            :, 4 * b: 4 * b + 4, :,
                        ],
                        o_sb.rearrange("d (c n) -> d c n", n=512),
                    )
                # ---- all-to-all for this head-phase ----
                nc.gpsimd.collective_compute(
                    "AllToAll",
                    mybir.AluOpType.bypass,
                    replica_groups=[list(range(W))],
                    ins=[a2a_in[h].opt()],
                    outs=[a2a_out[h].opt()],
                )
                # core j's chunk holds head 2j+h = channel rows 128j + 64h + d;
                # one DMA per kt so the kt-outer proj half starts on the
                # first arrived chunk instead of after the full 0.5MB load
                for j in range(W):
                    nc.sync.dma_start(
                        rx_sb[h][64 * h: 64 * h + D, j, :],
                        a2a_out[h].rearrange("(j d) n -> d j n", d=D)[:, j, :],
                    )

            # ---- phase 3: projection for this core's (b, chunk) ----
            # first-half matmuls depend only on a2a #0, so they run while
            # a2a #1 is still in flight
            phase_ctx.close()
            pspj = phase_ctx.enter_context(
                tc.tile_pool(name="psumproj", bufs=1, space="PSUM"))
            ps_pj = pspj.tile([P, KT, 512], F32, tag="pj")
            for mt in range(KT):
                for kt in range(KT):
                    nc.tensor.matmul(
                        ps_pj[:, mt, :],
                        wp_sb[:, kt, P * mt: P * (mt + 1)], rx_sb[0][:, kt],
                        start=(kt == 0), stop=False,
                    )
            for kt in range(KT):
                for mt in range(KT):
                    nc.tensor.matmul(
                        ps_pj[:, mt, :],
                        wp_sb[:, kt, P * mt: P * (mt + 1)], rx_sb[1][:, kt],
                        start=False, stop=(kt == KT - 1),
                    )
            for mt in range(KT):
                o_sb = wk_pool.tile([P, 512], BF16, tag="proj")
                nc.vector.tensor_scalar_add(o_sb[:], ps_pj[:, mt, :], bp_sb[:, mt: mt + 1])
                nc.sync.dma_start(out[P * mt: P * (mt + 1), :], o_sb[:])

    _split_multiwait(nc, gate_sems)
    return nc


_NC_CACHE: bass.Bass | None = None


def _get_nc() -> bass.Bass:
    global _NC_CACHE
    if _NC_CACHE is None:
        _NC_CACHE = _build_nc()
    return _NC_CACHE


def _prep_inputs(x, qkv_w, qkv_b, proj_w, proj_b):
    x = np.asarray(x, dtype=np.float32)
    qkv_w = np.asarray(qkv_w, dtype=np.float32)
    qkv_b = np.asarray(qkv_b, dtype=np.float32)
    proj_w = np.asarray(proj_w, dtype=np.float32)
    proj_b = np.asarray(proj_b, dtype=np.float32)

    # x.T pre-tiled as [chunk j, partition p, ktile, col] so each DMA
    # descriptor is one contiguous 8KB partition row
    xT2 = np.concatenate([x[b].T for b in range(B)], axis=1)  # [C, B*N]
    xT = np.ascontiguousarray(
        xT2.reshape(KT, P, B * NQC, 512).transpose(2, 1, 0, 3)
    ).astype(BF)
    wp = np.ascontiguousarray(
        (-proj_w).reshape(KT, P, C).transpose(1, 0, 2)).astype(BF)
    bp = np.ascontiguousarray(proj_b.reshape(KT, P).T)  # [p, mtile]

    in_maps = []
    for i in range(W):
        ch0 = P * i  # first channel of this core's head pair
        def wslice(col0):
            w = qkv_w[:, col0: col0 + P].reshape(KT, P, P).transpose(1, 0, 2)
            return np.ascontiguousarray(w).astype(BF)
        wq_i = wslice(ch0)
        wk_i = wslice(C + ch0)
        wv_i = wslice(2 * C + ch0)
        bq_i = np.ascontiguousarray(
            (qkv_b[ch0: ch0 + P] * SCALE).reshape(P, 1)
        )
        bk_i = np.ascontiguousarray(qkv_b[C + ch0: C + ch0 + P].reshape(P, 1))
        bv_i = np.ascontiguousarray(
            np.broadcast_to(qkv_b[2 * C + ch0: 2 * C + ch0 + P], (P, P))
        )
        in_maps.append({
            "xT": xT, "wq": wq_i, "wk": wk_i, "wv": wv_i,
            "bq": bq_i, "bk": bk_i, "bvr": bv_i,
            "wp": wp, "bp": bp,
        })
    return in_maps


def kernel(x, qkv_w, qkv_b, proj_w, proj_b, _trace=False, _trace_kwargs=None):
    nc = _get_nc()
    in_maps = _prep_inputs(x, qkv_w, qkv_b, proj_w, proj_b)
    res = run_bass_kernel_spmd(
        nc, in_maps, list(range(W)), trace=_trace, **(_trace_kwargs or {})
    )
    out = np.empty((B, N, C), dtype=np.float32)
    for i in range(W):
        b, g = i // 4, i % 4
        out[b, 512 * g: 512 * (g + 1), :] = \
            res.results[i]["out"].astype(np.float32).T
    kernel._last_result = res
    return out


# revision 38
# speedup vs baseline: 1.0267x; 1.0267x over previous
"""Multi-head self-attention (B=2, N=2048, C=1024, H=16) on 8 trn2 NeuronCores.

Sharding: core i computes heads {2i, 2i+1} for both batches (head-parallel
attention); a head-split pair of 8-way AllToAlls redistributes attention
outputs so core i holds the full channel dim for output rows
(b = i//4, seq chunk 512*(i%4)); each core then computes its own 512-row
slice of the output projection.

All matmuls present a full 128x128 stationary tile to the PE array so the
HAM clock gate holds the 2.4 GHz state (half-array matmuls run at 1.2 GHz):
 - scores: k is stored zero-spread per head (kTz{h} has head h's 64 rows,
   zeros elsewhere) so K=128 against the packed q tile adds 0*q_other.
 - att@v: stationary is [v_h | 64 ones columns]; psum rows 64:128 come out
   as the softmax denominator replicated across 64 partitions; one copy
   releases the accumulator and the reciprocal runs on a DMA-packed
   [64,32] repack (a flat [64,2048] DVE reciprocal costs 13us).
The exp(scores) stream on the Scalar engine is the critical path; the nk
loop emits next-iteration score matmuls between att@v matmuls so ACT never
waits on PE.
"""

import numpy as np
import ml_dtypes
import bass_rust

import concourse.bass as bass
import concourse.mybir as mybir
import concourse.tile as tile
from concourse.bass_utils import run_bass_kernel_spmd

B, N, C = 2, 2048, 1024
H = 16
D = C // H           # 64
W = 8                # cores
HL = 2               # heads per core
P = 128
KT = C // P          # 8 k-tiles over channels
NQC = N // 512       # 4 query chunks of 512 per batch
NK = N // P          # 16 key tiles per batch
SCALE = float(D) ** -0.5

F32 = mybir.dt.float32
BF16 = mybir.dt.bfloat16
BF = ml_dtypes.bfloat16


_RING_INSTS = (
    mybir.InstDMACopy, mybir.InstDMA, mybir.InstTensorLoad, mybir.InstTensorSave,
    mybir.InstDmaTransposeAnt, mybir.InstDMAGatherAnt, mybir.InstDMAScatterAddAnt,
    mybir.InstCollectiveCompute,
)


def _split_multiwait(nc: bass.Bass, gate_sems: dict) -> None:
    """This toolchain's walrus codegen accepts at most ONE sync wait per
    instruction, but the Tile scheduler attaches several.

    Compute/CTRL instructions: move all but the last wait onto EventSemaphore
    instructions inserted just before them on the same engine stream (engine
    sequencers execute in order, so the stall transfers).

    DMA / collective instructions are processed by the DGE ring / TOPSP, which
    a preceding stream stall does not reliably gate. For those, the inserted
    EventSemaphores absorb ALL original waits and the last one increments a
    per-engine gate semaphore; the ring instruction then carries the single
    gate wait."""
    ctr = 0
    counts: dict[int, int] = {}
    for fn in nc.m.functions:
        for bb in fn.blocks:
            out = []
            changed = False
            for ins in bb.instructions:
                si = ins.sync_info
                if si is None or len(si.on_wait) <= 1:
                    out.append(ins)
                    continue
                changed = True
                waits = list(si.on_wait)
                eng = ins.engine
                if isinstance(ins, _RING_INSTS):
                    h = gate_sems[eng]
                    cnt = counts.get(h.num, 0) + 1
                    counts[h.num] = cnt
                    for j, w in enumerate(waits):
                        ctr += 1
                        ev = mybir.InstEventSemaphore(
                            name=f"gate-ev-{ctr}", engine=eng)
                        upd = []
                        if j == len(waits) - 1:
                            upd = [bass_rust.SyncUpdate(
                                sync_type="semaphore", id=h.num, ant_name=h.name,
                                update_mode="sem-inc", update_value=1,
                                update_reg=None)]
                        ev.sync_info = bass_rust.SyncInfo(on_wait=[w], on_update=upd)
                        out.append(ev)
                    ins.sync_info = bass_rust.SyncInfo(
                        on_wait=[bass_rust.SyncWait(
                            sync_type="semaphore", id=h.num, ant_name=h.name,
                            wait_mode="sem-ge-imm", wait_value=cnt,
                            wait_reg=None)],
                        on_update=list(si.on_update),
                    )
                else:
                    for w in waits[:-1]:
                        ctr += 1
                        ev = mybir.InstEventSemaphore(
                            name=f"gate-ev-{ctr}", engine=eng)
                        ev.sync_info = bass_rust.SyncInfo(on_wait=[w], on_update=[])
                        out.append(ev)
                    ins.sync_info = bass_rust.SyncInfo(
                        on_wait=[waits[-1]], on_update=list(si.on_update)
                    )
                out.append(ins)
            if changed:
                bb.instructions = out


def _build_nc() -> bass.Bass:
    nc = bass.Bass()
    gate_sems = {
        e: nc.alloc_semaphore(f"mw_gate_{i}")
        for i, e in enumerate([
            mybir.EngineType.SP, mybir.EngineType.Pool,
            mybir.EngineType.Activation, mybir.EngineType.PE,
            mybir.EngineType.DVE,
        ])
    }

    # DRAM parameters (bf16 compute inputs prepared host-side)
    xT = nc.declare_dram_parameter("xT", [B * NQC, P, KT, 512], BF16, isOutput=False)
    # qkv/proj weights pre-transposed host-side to the SBUF layout
    wq = nc.declare_dram_parameter("wq", [P, KT, P], BF16, isOutput=False)
    wk = nc.declare_dram_parameter("wk", [P, KT, P], BF16, isOutput=False)
    wv = nc.declare_dram_parameter("wv", [P, KT, P], BF16, isOutput=False)
    bq = nc.declare_dram_parameter("bq", [P, 1], F32, isOutput=False)   # pre-scaled
    bk = nc.declare_dram_parameter("bk", [P, 1], F32, isOutput=False)
    bvr = nc.declare_dram_parameter("bvr", [P, P], F32, isOutput=False)  # replicated
    wp = nc.declare_dram_parameter("wp", [P, KT, C], BF16, isOutput=False)
    bp = nc.declare_dram_parameter("bp", [P, KT], F32, isOutput=False)  # [p, mtile]
    out = nc.declare_dram_parameter("out", [C, 512], BF16, isOutput=True)

    from contextlib import ExitStack
    with tile.TileContext(nc) as tc:
        with (
            tc.tile_pool(name="persist", bufs=1) as pp,
            tc.tile_pool(name="work", bufs=3) as wk_pool,
            tc.tile_pool(name="norm", bufs=2) as norm_pool,
            tc.tile_pool(name="dram", bufs=1, space="DRAM") as dram,
            ExitStack() as phase_ctx,
        ):
            psp = phase_ctx.enter_context(
                tc.tile_pool(name="psum", bufs=2, space="PSUM"))
            import contextlib
            psq_ctx = contextlib.ExitStack()
            psq = psq_ctx.enter_context(
                tc.tile_pool(name="psumq", bufs=1, space="PSUM"))
            # ---- persistent SBUF loads ----
            xT_t = [pp.tile([P, KT, 512], BF16, tag=f"xT{j}", name=f"xT_{j}")
                    for j in range(B * NQC)]

            def load_x_chunk(j, splits=1):
                # quarter-split early chunks so their descriptors spread
                # over several DMA queues and land sooner
                step = KT // splits
                for s in range(splits):
                    nc.sync.dma_start(
                        xT_t[j][:, s * step: (s + 1) * step, :],
                        xT[j][:, s * step: (s + 1) * step, :])

            # qkv weights arrive host-pre-transposed as [p, kt*m] so each
            # DMA is 128 descriptors of one contiguous 2KB partition row
            # (the on-the-fly rearrange costs 1024 x 256B descriptors).
            # xT[0] is issued first: it is 4x larger than wq and equally
            # critical for the first matmul.
            load_x_chunk(0, splits=4)
            wq_sb = pp.tile([P, KT, P], BF16, tag="wq")
            nc.sync.dma_start(wq_sb[:], wq[:])
            bq_sb = pp.tile([P, 1], F32, tag="bq")
            nc.sync.dma_start(bq_sb[:], bq[:])
            load_x_chunk(1, splits=4)
            wk_sb = pp.tile([P, KT, P], BF16, tag="wk")
            nc.sync.dma_start(wk_sb[:], wk[:])
            bk_sb = pp.tile([P, 1], F32, tag="bk")
            nc.sync.dma_start(bk_sb[:], bk[:])
            wv_sb = pp.tile([P, KT, P], BF16, tag="wv")
            nc.sync.dma_start(wv_sb[:], wv[:])
            bvr_sb = pp.tile([P, P], F32, tag="bvr")
            nc.sync.dma_start(bvr_sb[:], bvr[:])
            # preload the exp activation table during the DMA phase: the lazy
            # ACT_TABLE_LOAD (~2.7us) otherwise lands inside the attention
            # pipeline fill
            warm_exp = pp.tile([P, 1], F32, tag="warm_exp")
            nc.scalar.activation(
                warm_exp[:], bq_sb[:], mybir.ActivationFunctionType.Exp)

            # persistent activations, one tile per (batch, 512-chunk) so
            # consumers wait only on the chunk they read.  q is packed
            # (head 2i rows 0:64, head 2i+1 rows 64:128); k is stored
            # zero-spread per head so score matmuls can contract K=128.
            qT_t = [pp.tile([P, 512], BF16, tag=f"qT{j}", name=f"qT_{j}")
                    for j in range(B * NQC)]
            kTz_t = [
                [pp.tile([P, 512], BF16, tag=f"kTz{h}_{j}", name=f"kTz{h}_{j}")
                 for j in range(B * NQC)]
                for h in range(HL)
            ]
            # v_ext: [seq128, b, seqtile, head, 128]; cols 0:D hold v, cols
            # D:128 are ones so the av matmul is a full 128-wide stationary
            # operand and psum rows D:128 accumulate the softmax denominator
            # (replicated across 64 partitions).
            v_sb = pp.tile([P, B, NK, HL, P], BF16, tag="v")
            # proj inputs, one per a2a phase; unfilled head-half stays zero
            rx_sb = [pp.tile([P, KT, 512], BF16, tag=f"rx{h}", name=f"rx_{h}")
                     for h in range(HL)]

            a2a_in = [dram.tile([W * D, 512], BF16, tag=f"a2a_in_{h}",
                                name=f"a2a_in_{h}") for h in range(HL)]
            a2a_out = [dram.tile([W * D, 512], BF16, tag=f"a2a_out_{h}",
                                 name=f"a2a_out_{h}") for h in range(HL)]

            # ---- phase 1: qkv ----
            load_x_chunk(2, splits=2)
            load_x_chunk(3, splits=2)
            for j in range(4, B * NQC):
                load_x_chunk(j)
            # proj weights are only needed at the end — load them after the
            # critical-path xT chunks (same FIFO DMA ring)
            wp_sb = pp.tile([P, KT, C], BF16, tag="wp")
            nc.sync.dma_start(wp_sb[:], wp[:])
            bp_sb = pp.tile([P, KT], F32, tag="bp")
            nc.sync.dma_start(bp_sb[:], bp[:])
            # j-outer accumulation: each (b, chunk) finishes its 8-kt psum
            # accumulation then drains while the next chunk's matmuls run.
            # psum tags are laid out so no chunk ever waits a drain: q on
            # tags 0-7, k/v rotate through 4-7.
            def emit_q(j):
                ps = psq.tile([P, 512], F32, tag=f"qk{'AB'[j % 2]}", name=f"ps_q_{j}")
                for kt in range(KT):
                    nc.tensor.matmul(
                        ps[:], wq_sb[:, kt], xT_t[j][:, kt, :],
                        start=(kt == 0), stop=(kt == KT - 1),
                    )
                nc.vector.tensor_scalar(
                    qT_t[j][:], ps[:],
                    SCALE, bq_sb[:], mybir.AluOpType.mult, mybir.AluOpType.add,
                )

            def emit_k(j, tag):
                ps = psq.tile([P, 512], F32, tag=tag, name=f"ps_k_{j}")
                for kt in range(KT):
                    nc.tensor.matmul(
                        ps[:], wk_sb[:, kt], xT_t[j][:, kt, :],
                        start=(kt == 0), stop=(kt == KT - 1),
                    )
                nc.vector.tensor_scalar_add(
                    kTz_t[0][j][0:D, :], ps[0:D, :], bk_sb[0:D],
                )
                nc.vector.tensor_scalar_add(
                    kTz_t[1][j][D:P, :], ps[D:P, :], bk_sb[D:P],
                )
                nc.vector.memset(kTz_t[0][j][D:P, :], 0.0)
                nc.vector.memset(kTz_t[1][j][0:D, :], 0.0)

            def emit_v(b, st):
                tag = f"qk{'AB'[st % 2]}"
                ps = psq.tile([P, P], F32, tag=tag, name=f"ps_v_{b}_{st}")
                xt = xT_t[4 * b + st // 4]
                so = P * (st % 4)
                for kt in range(KT):
                    nc.tensor.matmul(
                        ps[:], xt[:, kt, so: so + P], wv_sb[:, kt],
                        start=(kt == 0), stop=(kt == KT - 1),
                    )
                nc.vector.tensor_tensor(
                    v_sb[:, b, st, :, 0:D],
                    ps.rearrange("p (h d) -> p h d", h=HL),
                    bvr_sb.rearrange("p (h d) -> p h d", h=HL),
                    mybir.AluOpType.add,
                )

            # ---- attention emission helpers (psp is already open, below
            # the qkv rings, so the first head-phase can pre-run scores+exp
            # on the idle ACT engine while late qkv still computes) ----
            def emit_scores_half(b, h, nk, t):
                """Scores for key tile nk, query chunks 2t..2t+1 -> one
                [128, 1024] psum tile.  K=128 against packed q: the zero
                rows of kTz contribute 0 * q_other."""
                ps = psp.tile([P, 1024], F32, tag="s", name=f"ps_s_{b}_{h}_{nk}_{t}")
                kz = kTz_t[h][4 * b + nk // 4]
                ko = P * (nk % 4)
                for i in range(2):
                    c = 2 * t + i
                    nc.tensor.matmul(
                        ps[:, 512 * i: 512 * (i + 1)],
                        kz[:, ko: ko + P],
                        qT_t[4 * b + c][:],
                        start=True, stop=True,
                    )
                return ps

            EXP_BUFS = 14
            PD = 7  # (h0,b0) score/exp pre-run depth in key tiles

            def emit_exp(b, h, nk, t):
                ps = emit_scores_half(b, h, nk, t)
                e = wk_pool.tile([P, 1024], BF16, tag="exp", bufs=EXP_BUFS,
                                 name=f"exp_{b}_{h}_{nk}_{t}")
                nc.scalar.activation(
                    e[:], ps[:], mybir.ActivationFunctionType.Exp)
                return e

            pre_exps = {}

            def pre_nk(nk):
                for t in range(2):
                    pre_exps[(nk, t)] = emit_exp(0, 0, nk, t)

            for j in range(8):
                emit_q(j)
            for j in range(4):
                emit_k(j, f"qk{'CD'[j % 2]}")
            nc.vector.memset(v_sb[:, :, :, :, D:P], 1.0)
            for st in range(NK):
                emit_v(0, st)
            for j in range(4, 8):
                emit_k(j, f"qk{'CD'[j % 2]}")
                pre_nk(j - 4)
            for st in range(NK):
                emit_v(1, st)
                if st < PD - 4:
                    pre_nk(4 + st)
            # qkv psum rings retire; ps_o takes their banks
            psq_ctx.close()
            psp2 = phase_ctx.enter_context(
                tc.tile_pool(name="psum2", bufs=1, space="PSUM"))

            nc.vector.memset(rx_sb[0][D:P, :, :], 0.0)
            nc.vector.memset(rx_sb[1][0:D, :, :], 0.0)

            # ---- phase 2: attention, h-outer so each head-phase feeds an
            # AllToAll that overlaps the next phase's compute ----
            for h in range(HL):
                for b in range(B):
                    ps_o = psp2.tile([P, N], F32, tag="o", name=f"ps_o_{b}_{h}")
                    first = (h == 0 and b == 0)
                    pipe = pre_exps if first else {}
                    pd = PD if first else 1
                    if not first:
                        for t in range(2):
                            pipe[(0, t)] = emit_exp(b, h, 0, t)
                    for nk in range(NK):
                        for t in range(2):
                            if nk + pd < NK:
                                pipe[(nk + pd, t)] = emit_exp(b, h, nk + pd, t)
                            e = pipe.pop((nk, t))
                            for i in range(2):
                                c = 2 * t + i
                                nc.tensor.matmul(
                                    ps_o[:, 512 * c: 512 * (c + 1)],
                                    v_sb[:, b, nk, h],
                                    e[:, 512 * i: 512 * (i + 1)],
                                    start=(nk == 0), stop=(nk == NK - 1),
                                )
                    # normalize rows 0:D by the denominator in row D: one
                    # copy releases ps_o for the next (b,h); the reciprocal
                    # runs on a DMA-packed [64,32] repack (a flat [64,2048]
                    # DVE reciprocal costs 13us), then a broadcast read
                    # replicates it across partitions for the multiply.
                    o_raw = norm_pool.tile([D + 1, N], F32, tag="oraw",
                                           name=f"oraw_{b}_{h}")
                    d_dram = dram.tile([1, N], F32, tag=f"dd_{b}_{h}",
                                       name=f"dd_{b}_{h}")
                    nc.vector.tensor_copy(o_raw[:], ps_o[0: D + 1, :])
                    nc.sync.dma_start(d_dram[:], o_raw[D: D + 1, :])
                    rsc = norm_pool.tile([D, N // D], F32, tag="rsc",
                                         name=f"rsc_{b}_{h}")
                    nc.sync.dma_start(
                        rsc[:], d_dram.rearrange("o (p f) -> (o p) f", p=D))
                    rscr = norm_pool.tile([D, N // D], F32, tag="rscr",
                                          name=f"rscr_{b}_{h}")
                    nc.vector.reciprocal(rscr[:], rsc[:])
                    r_dram = dram.tile([D, N // D], F32, tag=f"rd_{b}_{h}",
                                       name=f"rd_{b}_{h}")
                    nc.sync.dma_start(r_dram[:], rscr[:])
                    rec = norm_pool.tile([D, N], F32, tag="rec", bufs=1,
                                         name=f"rec_{b}_{h}")
                    nc.sync.dma_start(
                        rec[:, None, :],
                        r_dram.rearrange("p f -> (p f)")[None, :].partition_broadcast(D))
                    o_sb = norm_pool.tile([D, N], BF16, tag="osb", name=f"osb_{b}_{h}")
                    nc.vector.tensor_tensor(
                        o_sb[:], o_raw[0:D, :], rec[:], mybir.AluOpType.mult,
                    )
                    nc.sync.dma_start(
                        a2a_in[h].rearrange("(j r) n -> r j n", r=D)[
                            :, 4 * b: 4 * b + 4, :,
                        ],
                        o_sb.rearrange("d (c n) -> d c n", n=512),
                    )
                # ---- all-to-all for this head-phase ----
                nc.gpsimd.collective_compute(
                    "AllToAll",
                    mybir.AluOpType.bypass,
                    replica_groups=[list(range(W))],
                    ins=[a2a_in[h].opt()],
                    outs=[a2a_out[h].opt()],
                )
                # core j's chunk holds head 2j+h = channel rows 128j + 64h + d;
                # one DMA per kt so the kt-outer proj half starts on the
                # first arrived chunk instead of after the full 0.5MB load
                for j in range(W):
                    nc.sync.dma_start(
                        rx_sb[h][64 * h: 64 * h + D, j, :],
                        a2a_out[h].rearrange("(j d) n -> d j n", d=D)[:, j, :],
                    )

            # ---- phase 3: projection for this core's (b, chunk) ----
            # first-half matmuls depend only on a2a #0, so they run while
            # a2a #1 is still in flight
            phase_ctx.close()
            pspj = phase_ctx.enter_context(
                tc.tile_pool(name="psumproj", bufs=1, space="PSUM"))
            ps_pj = pspj.tile([P, KT, 512], F32, tag="pj")
            for mt in range(KT):
                for kt in range(KT):
                    nc.tensor.matmul(
                        ps_pj[:, mt, :],
                        wp_sb[:, kt, P * mt: P * (mt + 1)], rx_sb[0][:, kt],
                        start=(kt == 0), stop=False,
                    )
            for kt in range(KT):
                for mt in range(KT):
                    nc.tensor.matmul(
                        ps_pj[:, mt, :],
                        wp_sb[:, kt, P * mt: P * (mt + 1)], rx_sb[1][:, kt],
                        start=False, stop=(kt == KT - 1),
                    )
            for mt in range(KT):
                o_sb = wk_pool.tile([P, 512], BF16, tag="proj")
                nc.vector.tensor_scalar_add(o_sb[:], ps_pj[:, mt, :], bp_sb[:, mt: mt + 1])
                nc.sync.dma_start(out[P * mt: P * (mt + 1), :], o_sb[:])

    _split_multiwait(nc, gate_sems)
    return nc


_NC_CACHE: bass.Bass | None = None


def _get_nc() -> bass.Bass:
    global _NC_CACHE
    if _NC_CACHE is None:
        _NC_CACHE = _build_nc()
    return _NC_CACHE


def _prep_inputs(x, qkv_w, qkv_b, proj_w, proj_b):
    x = np.asarray(x, dtype=np.float32)
    qkv_w = np.asarray(qkv_w, dtype=np.float32)
    qkv_b = np.asarray(qkv_b, dtype=np.float32)
    proj_w = np.asarray(proj_w, dtype=np.float32)
    proj_b = np.asarray(proj_b, dtype=np.float32)

    # x.T pre-tiled as [chunk j, partition p, ktile, col] so each DMA
    # descriptor is one contiguous 8KB partition row
    xT2 = np.concatenate([x[b].T for b in range(B)], axis=1)  # [C, B*N]
    xT = np.ascontiguousarray(
        xT2.reshape(KT, P, B * NQC, 512).transpose(2, 1, 0, 3)
    ).astype(BF)
    wp = np.ascontiguousarray(
        proj_w.reshape(KT, P, C).transpose(1, 0, 2)).astype(BF)
    bp = np.ascontiguousarray(proj_b.reshape(KT, P).T)  # [p, mtile]

    in_maps = []
    for i in range(W):
        ch0 = P * i  # first channel of this core's head pair
        def wslice(col0):
            w = qkv_w[:, col0: col0 + P].reshape(KT, P, P).transpose(1, 0, 2)
            return np.ascontiguousarray(w).astype(BF)
        wq_i = wslice(ch0)
        wk_i = wslice(C + ch0)
        wv_i = wslice(2 * C + ch0)
        bq_i = np.ascontiguousarray(
            (qkv_b[ch0: ch0 + P] * SCALE).reshape(P, 1)
        )
        bk_i = np.ascontiguousarray(qkv_b[C + ch0: C + ch0 + P].reshape(P, 1))
        bv_i = np.ascontiguousarray(
            np.broadcast_to(qkv_b[2 * C + ch0: 2 * C + ch0 + P], (P, P))
        )
        in_maps.append({
            "xT": xT, "wq": wq_i, "wk": wk_i, "wv": wv_i,
            "bq": bq_i, "bk": bk_i, "bvr": bv_i,
            "wp": wp, "bp": bp,
        })
    return in_maps


def kernel(x, qkv_w, qkv_b, proj_w, proj_b, _trace=False, _trace_kwargs=None):
    nc = _get_nc()
    in_maps = _prep_inputs(x, qkv_w, qkv_b, proj_w, proj_b)
    res = run_bass_kernel_spmd(
        nc, in_maps, list(range(W)), trace=_trace, **(_trace_kwargs or {})
    )
    out = np.empty((B, N, C), dtype=np.float32)
    for i in range(W):
        b, g = i // 4, i % 4
        out[b, 512 * g: 512 * (g + 1), :] = \
            res.results[i]["out"].astype(np.float32).T
    kernel._last_result = res
    return out
